# revision 1
# baseline (speedup 1.0000x reference)
"""Causal self-attention (B=4, T=2048, C=1024, H=16, D=64) on 8 trn2 cores.

Sharding: core i handles batch b = i//2 and head-group g = i%2 (8 of 16
heads), tensor-parallel over c_attn columns / c_proj rows. Each core
computes qkv for its heads, causal attention, and a partial projection
(its 512 rows of w_proj); the host sums the two partials per batch and
adds b_proj.

Per-core device pipeline (feature-major layouts avoid all transposes):
  xT [C, T] (host-transposed; bf16 for q/k via in-flight DMA cast, f32r
  for the v path)  --matmul-->  qkT [1024, T] feature-major (bf16)
                   --matmul-->  v [T, 8*(64+1)] token-major (+ones cols)
  S^T strips [j, i] = kT_h.T @ qT_h (K=64, fp32 psum), exp on ACT with
  scale=1/8, strips trimmed to the exact causal start (jc*128); one
  [128,128] upper-tri mask multiply on the diagonal block (widened to
  256 cols on jc%4==3 strips so every matmul has N>=256),
  yT_aug [65, i] += v_aug.T @ expS (row 64 = softmax denominators, via
  the ones column), normalize with reciprocal + gpsimd
  partition_broadcast, out[t, o] = sum_l yT[l, t].T @ w_p[l, o].

Scheduling: emission order is per-engine execution order, so attention
strips (ACT-heavy) are interleaved with qkv/projection matmul groups
(PE-heavy) through a demand-driven filler queue; attention head-passes
start as soon as their qk feature chunks land. PSUM: psA 2 + psS 2x2 +
psY 2 = 8 banks.
"""

import sys

sys.path.insert(0, "/opt/trn_rl_repo")

from collections import deque
from contextlib import ExitStack

import ml_dtypes
import numpy as np

import concourse.bass as bass
import concourse.mybir as mybir
import concourse.tile as tile
from concourse import bacc
from concourse import bass_utils

f32 = mybir.dt.float32
f32r = mybir.dt.float32r
bf16 = mybir.dt.bfloat16
EXP = mybir.ActivationFunctionType.Exp
MUL = mybir.AluOpType.mult
ADD = mybir.AluOpType.add

B, T, C, H, D = 4, 2048, 1024, 16, 64
HL = H // 2          # 8 heads per core
CL = HL * D          # 512 local feature width
P = 128
KC = C // P          # 8 contraction chunks over C
NJC = T // P         # 16 token chunks of 128
NIC = T // 512       # 4 i-chunks of 512
NTC = T // 512       # 4 t-chunks of 512 in phase A
JQK = 2 * CL // P    # 8 qk feature chunks of 128


def build_body(tc, aps):
    nc = tc.nc
    wqk, bqk, wv, bv, wp, masks, outp = (
        aps["wqk"], aps["bqk"], aps["wv"], aps["bv"],
        aps["wp"], aps["masks"], aps["outp"],
    )

    with ExitStack() as ctx:
        const = ctx.enter_context(tc.tile_pool(name="const", bufs=1))
        qkT_pool = ctx.enter_context(tc.tile_pool(name="qkT", bufs=1))
        vaug_pool = ctx.enter_context(tc.tile_pool(name="vaug", bufs=1))
        yT_pool = ctx.enter_context(tc.tile_pool(name="yT", bufs=1))

        masks_sb = const.tile([P, P], f32)
        nc.sync.dma_start(masks_sb[:], masks[:])
        mask2_sb = const.tile([P, 2 * P], f32)
        nc.sync.dma_start(mask2_sb[:], aps["mask2"][:])
        bqk_sb = const.tile([P, JQK], f32)
        nc.sync.dma_start(bqk_sb[:], bqk.rearrange("(j p) -> p j", p=P))
        bv_rep = const.tile([P, CL], f32)
        nc.sync.dma_start(bv_rep[:], bv[None, :].to_broadcast([P, CL]))

        qkT = qkT_pool.tile([P, JQK, T], bf16)
        vaug = vaug_pool.tile([P, NJC, HL, D + 1], f32r)
        nc.vector.memset(
            vaug[:, :, :, D : D + 1].bitcast(mybir.dt.uint32), 0x3F800000
        )

        # ---------------- Phase A/B interleaved.
        # A1 builds v + qk for head-pairs 0,1; B(0..3) (ACT-heavy) then
        # overlaps A2 (PE-heavy qk for pairs 2,3, re-streaming xT);
        # B(4..7) follows. PSUM: psA 2 + psS 4 + psY 2 = 8 banks.
        def qk_chunk_thunks(j, xT_sb):
            # one thunk per (chunk, tci) psum group; first thunk loads wq
            wq_box = {}

            def load(tci):
                if tci == 0:
                    wq_t = wq_pool.tile([P, KC, P], bf16, tag="wq", name=f"wq{j}")
                    nc.sync.dma_start(wq_t[:], wqk[j])
                    wq_box["t"] = wq_t
                wq_t = wq_box["t"]
                ps = psA.tile([P, 512], f32, tag="a")
                for k in range(KC):
                    nc.tensor.matmul(
                        ps[:], wq_t[:, k, :],
                        xT_sb[:, k, tci * 512 : tci * 512 + 512],
                        start=(k == 0), stop=(k == KC - 1),
                    )
                nc.scalar.activation(
                    qkT[:, j, tci * 512 : tci * 512 + 512], ps[:],
                    mybir.ActivationFunctionType.Identity,
                    bias=bqk_sb[:, j : j + 1],
                )

            return [lambda tci=tci: load(tci) for tci in range(NTC)]

        def v_chunk_thunks(xT_sb, wv_sb):
            xTv_r = aps["xTv"].rearrange("(k p) t -> p k t", p=P)

            def vchunk(tci128):
                xtv = xtv_pool.tile([P, KC, P], f32r, tag="xtv")
                nc.sync.dma_start(
                    xtv[:], xTv_r[:, :, tci128 * P : (tci128 + 1) * P]
                )
                ps = psA.tile([P, 512], f32, tag="a")
                for k in range(KC):
                    nc.tensor.matmul(
                        ps[:],
                        xtv[:, k, :],
                        wv_sb[:, k, :],
                        start=(k == 0), stop=(k == KC - 1),
                    )
                nc.vector.tensor_tensor(
                    vaug[:, tci128, :, 0:D],
                    ps[:].rearrange("p (h d) -> p h d", h=HL),
                    bv_rep[:].rearrange("p (h d) -> p h d", h=HL), ADD,
                )

            return [lambda t=t: vchunk(t) for t in range(NJC)]

        def emit_b_pass(h, pp, drain):
            pr, half = h // 2, h % 2
            base = half * 64
            qTh = qkT[base : base + 64, pr, :]
            kTh = qkT[base : base + 64, 4 + pr, :]

            if True:
                lo = pp * 1024
                yt_ps = [
                    psY.tile([P, 512], f32, tag="yt", name=f"yt{h}_{pp}_{i}")
                    for i in range(2)
                ]
                for jc in range(8 * pp + 8):
                    if h < 2:
                        need(jc + 1)
                    else:
                        need(16 + 8 * (h // 2))
                        if pp == 1 and h >= 2:
                            # spread the first-half projection tiles across
                            # the late ACT-bound head passes
                            need(40 + min(16, (h - 2) * 3 + jc // 6))
                    drain()
                    diag = jc * P >= lo
                    # widen width-128 diagonal strips to 256 (fp32r runs at
                    # 4 cyc/row below N=256); the extra 128 cols are zeroed
                    # by the wider mask.
                    wide = diag and jc % 4 == 3
                    start = max(jc * P - (P if wide else 0), lo)
                    ps_s = psS.tile([P, 1024], f32, tag="s")
                    for icp in range(2):
                        ic = 2 * pp + icp
                        if (ic + 1) * 512 <= start:
                            continue
                        c0 = max(start, ic * 512)
                        nc.tensor.matmul(
                            ps_s[:, c0 - lo : (icp + 1) * 512],
                            kTh[:, jc * P : (jc + 1) * P],
                            qTh[:, c0 : (ic + 1) * 512],
                            start=True, stop=True,
                        )
                    w = lo + 1024 - start
                    es = es_pool.tile([P, 1024], f32r, tag="es")
                    nc.scalar.activation(
                        es[:, :w], ps_s[:, start - lo : 1024], EXP, scale=0.125
                    )
                    if wide:
                        nc.vector.tensor_tensor(
                            es[:, 0 : 2 * P], es[:, 0 : 2 * P], mask2_sb[:], MUL
                        )
                    elif diag:
                        nc.vector.tensor_tensor(
                            es[:, 0:P], es[:, 0:P], masks_sb[:], MUL
                        )
                    for icp in range(2):
                        ic = 2 * pp + icp
                        if (ic + 1) * 512 <= start:
                            continue
                        c0 = max(start, ic * 512)
                        nc.tensor.matmul(
                            yt_ps[icp][0 : D + 1, c0 - ic * 512 : 512],
                            vaug[:, jc, h, :],
                            es[:, c0 - start : c0 - start + 512 - (c0 - ic * 512)],
                            start=(jc == 0), stop=(jc == 4 * ic + 3),
                        )
                for icp in range(2):
                    ic = 2 * pp + icp
                    rec = nrm_pool.tile([1, 512], f32, tag="rec")
                    nc.vector.reciprocal(rec[0:1, :], yt_ps[icp][D : D + 1, :])
                    rep = nrm_pool.tile([64, 512], f32, tag="rep")
                    nc.gpsimd.partition_broadcast(rep[:], rec[0:1, :])
                    nc.vector.tensor_tensor(
                        yT[base : base + 64, pr, ic * 512 : ic * 512 + 512],
                        yt_ps[icp][0:64, :], rep[:], MUL,
                    )

        yT = yT_pool.tile([P, CL // P, T], f32r)
        with ExitStack() as actx:
            xt_pool = actx.enter_context(tc.tile_pool(name="xt", bufs=1))
            xtv_pool = actx.enter_context(tc.tile_pool(name="xtv", bufs=3))
            wq_pool = actx.enter_context(tc.tile_pool(name="wq", bufs=2))
            psA = actx.enter_context(tc.tile_pool(name="psA", bufs=2, space="PSUM"))

            filler = deque()
            _dr = {"n": 0, "num": 0, "den": 1, "done": 0}

            def drain():
                # pop fillers at num/den per strip so PE-side filler work
                # spreads into the ACT-bound stretch instead of front-loading
                _dr["n"] += _dr["num"]
                while filler and _dr["n"] >= _dr["den"]:
                    _dr["n"] -= _dr["den"]
                    filler.popleft()()
                    _dr["done"] += 1

            def need(k):
                # force-drain so producers (v chunks, qk chunks) are emitted
                # before the strips that read them
                while filler and _dr["done"] < k:
                    filler.popleft()()
                    _dr["done"] += 1

            with tc.tile_pool(name="wv", bufs=1) as wv_pool:
                xT_sb = xt_pool.tile([P, KC, T], bf16)
                xT_r = aps["xTv"].rearrange("(k p) t -> p k t", p=P)

                def xq_dma(q):
                    # gpsimd DMA casts f32 -> bf16 in flight
                    nc.gpsimd.dma_start(
                        xT_sb[:, :, q * 512 : (q + 1) * 512],
                        xT_r[:, :, q * 512 : (q + 1) * 512],
                    )

                wv_sb = wv_pool.tile([P, KC, CL], f32r)

                # pair-0 qk chunks emitted up front, interleaved with the
                # xT quarter loads so the PE starts ~5us in; everything
                # else goes through the filler queue between B strips.
                xq_dma(0)
                q0 = qk_chunk_thunks(0, xT_sb)
                q4 = qk_chunk_thunks(4, xT_sb)
                q0[0]()
                q4[0]()
                for q in (1, 2, 3):
                    xq_dma(q)
                    q0[q]()
                    q4[q]()
                nc.sync.dma_start(
                    wv_sb[:], wv.rearrange("(k p) n -> p k n", p=P)
                )
                filler.extend(v_chunk_thunks(xT_sb, wv_sb))
                for j in (1, 5):
                    filler.extend(qk_chunk_thunks(j, xT_sb))

                with ExitStack() as bctx:
                    es_pool = bctx.enter_context(tc.tile_pool(name="es", bufs=3))
                    nrm_pool = bctx.enter_context(tc.tile_pool(name="nrm", bufs=1))
                    ostg = bctx.enter_context(tc.tile_pool(name="ostg", bufs=4))
                    wp_pool = bctx.enter_context(tc.tile_pool(name="wp", bufs=1))
                    psS = bctx.enter_context(
                        tc.tile_pool(name="psS", bufs=2, space="PSUM")
                    )
                    psY = bctx.enter_context(
                        tc.tile_pool(name="psY", bufs=2, space="PSUM")
                    )

                    def c_tile(tcb, oc):
                        ps = psA.tile([P, 512], f32, tag="a")
                        for li in range(4):
                            nc.tensor.matmul(
                                ps[:],
                                yT[:, li, tcb * P : (tcb + 1) * P],
                                wp_sb[:, li, oc * 512 : oc * 512 + 512],
                                start=(li == 0), stop=(li == 3),
                            )
                        ot = ostg.tile([P, 512], f32, tag="o")
                        if tcb >= 8 or (tcb + oc) % 2 == 0:
                            nc.scalar.copy(ot[:], ps[:])
                        else:
                            nc.vector.tensor_copy(ot[:], ps[:])
                        nc.sync.dma_start(
                            outp[tcb * P : (tcb + 1) * P,
                                 oc * 512 : oc * 512 + 512],
                            ot[:],
                        )

                    emit_b_pass(0, 0, drain)
                    for j in (2, 6, 3, 7):
                        filler.extend(qk_chunk_thunks(j, xT_sb))
                    emit_b_pass(0, 1, drain)
                    emit_b_pass(1, 0, drain)
                    emit_b_pass(1, 1, drain)
                    emit_b_pass(2, 0, drain)
                    emit_b_pass(3, 0, drain)
                    emit_b_pass(4, 0, drain)
                    wp_sb = wp_pool.tile([P, 4, C], f32r, name="wp_sb")
                    nc.sync.dma_start(
                        wp_sb[:], wp.rearrange("(l p) n -> p l n", p=P)
                    )
                    emit_b_pass(5, 0, drain)
                    emit_b_pass(6, 0, drain)
                    emit_b_pass(7, 0, drain)
                    filler.extend(
                        lambda t=t, o=o: c_tile(t, o)
                        for t in range(8) for o in range(2)
                    )
                    for h in range(2, 8):
                        emit_b_pass(h, 1, drain)
                    while filler:
                        filler.popleft()()
                    for t in range(8, NJC):
                        for o in range(2):
                            c_tile(t, o)


_CACHE = {}


def build_nc():
    if "nc" in _CACHE:
        return _CACHE["nc"]
    nc = bacc.Bacc(
        "TRN2",
        target_bir_lowering=False,
        debug=False,
        enable_asserts=False,
        num_devices=8,
    )
    aps = {
        "wqk": nc.dram_tensor("wqk", [JQK, P, KC, P], bf16, kind="ExternalInput").ap(),
        "bqk": nc.dram_tensor("bqk", [2 * CL], f32, kind="ExternalInput").ap(),
        "wv": nc.dram_tensor("wv", [C, CL], f32r, kind="ExternalInput").ap(),
        "xTv": nc.dram_tensor("xTv", [C, T], f32r, kind="ExternalInput").ap(),
        "bv": nc.dram_tensor("bv", [CL], f32, kind="ExternalInput").ap(),
        "wp": nc.dram_tensor("wp", [CL, C], f32r, kind="ExternalInput").ap(),
        "masks": nc.dram_tensor("masks", [P, P], f32, kind="ExternalInput").ap(),
        "mask2": nc.dram_tensor("mask2", [P, 2 * P], f32, kind="ExternalInput").ap(),
        "outp": nc.dram_tensor("outp", [T, C], f32, kind="ExternalOutput").ap(),
    }
    with tile.TileContext(nc) as tc:
        build_body(tc, aps)
    nc.compile()
    _CACHE["nc"] = nc
    return nc


def make_masks():
    # mask[jp, c] = 1 where column c (global i = jc*128 + c) >= row jp (j)
    return np.triu(np.ones((P, P), dtype=np.float32))


def make_mask2():
    m = np.zeros((P, 2 * P), dtype=np.float32)
    m[:, P:] = np.triu(np.ones((P, P), dtype=np.float32))
    return m


def make_in_maps(x, w_attn, b_attn, w_proj, b_proj):
    masks = make_masks()
    mask2 = make_mask2()
    in_maps = []
    xTs = [np.ascontiguousarray(x[b].T) for b in range(B)]
    for core in range(8):
        b, g = core // 2, core % 2
        xT_f32 = xTs[b]
        qcols = slice(g * CL, (g + 1) * CL)
        kcols = slice(C + g * CL, C + (g + 1) * CL)
        vcols = slice(2 * C + g * CL, 2 * C + (g + 1) * CL)
        in_maps.append(
            {
                "wqk": np.ascontiguousarray(
                    np.concatenate([w_attn[:, qcols], w_attn[:, kcols]], axis=1)
                    .astype(ml_dtypes.bfloat16)
                    .reshape(KC, P, JQK, P)
                    .transpose(2, 1, 0, 3)
                ),
                "bqk": np.ascontiguousarray(
                    np.concatenate([b_attn[qcols], b_attn[kcols]])
                ),
                "wv": np.ascontiguousarray(w_attn[:, vcols]),
                "xTv": xT_f32,
                "bv": np.ascontiguousarray(b_attn[vcols]),
                "wp": np.ascontiguousarray(w_proj[g * CL : (g + 1) * CL, :]),
                "masks": masks,
                "mask2": mask2,
            }
        )
    return in_maps


def combine(parts, b_proj):
    return np.stack(
        [parts[2 * b] + parts[2 * b + 1] + b_proj[None, :] for b in range(B)]
    ).astype(np.float32)


def kernel(x, w_attn, b_attn, w_proj, b_proj, _trace=False, **run_kwargs):
    x = np.asarray(x, dtype=np.float32)
    w_attn = np.asarray(w_attn, dtype=np.float32)
    b_attn = np.asarray(b_attn, dtype=np.float32)
    w_proj = np.asarray(w_proj, dtype=np.float32)
    b_proj = np.asarray(b_proj, dtype=np.float32)

    nc = build_nc()
    in_maps = make_in_maps(x, w_attn, b_attn, w_proj, b_proj)
    try:
        res = bass_utils.run_bass_kernel_spmd(
            nc, in_maps, core_ids=list(range(8)), trace=_trace, **run_kwargs
        )
    except Exception:
        # transient NRT device wedge: one retry
        res = bass_utils.run_bass_kernel_spmd(
            nc, in_maps, core_ids=list(range(8)), trace=_trace, **run_kwargs
        )
    parts = [res.results[i]["outp"] for i in range(8)]
    out = combine(parts, b_proj)
    if _trace:
        return out, res
    return out



# revision 19
# speedup vs baseline: 1.1289x; 1.1289x over previous
"""Causal self-attention (B=4, T=2048, C=1024, H=16, D=64) on 8 trn2 cores.

Sharding: core i handles batch b = i//2 and head-group g = i%2 (8 of 16
heads), tensor-parallel over c_attn columns / c_proj rows. Each core
computes qkv for its heads, causal attention, and a partial projection
(its 512 rows of w_proj); the host sums the two partials per batch and
adds b_proj.

v2 pipeline (per core), built around the engine cost model:
  - q/k/v projections run on the PE in fp8 DoubleRow mode with a hi/lo
    split (x = x_hi + x_lo, w = w_hi + w_lo, three cross terms): 256-wide
    contraction per instruction at 0.5 cyc/col = 2.67x bf16 throughput
    at ~bf16 accuracy.
  - q/k are stored to SBUF as fp8 (e4m3) in a [64d, 2, T] pair-plane
    layout whose second plane is zeroed; S^T strips then also run in
    DoubleRow mode (2 cyc per 4 cols) despite the 64-deep contraction.
  - exp on ACT (the critical engine: ~139k cols x 0.83ns) writes bf16
    es strips; the causal diagonal block is masked by a DVE multiply.
  - AV is token-major: es strip blocks [128j, 128i] are the *stationary*
    operand, v_aug [128j, 65] (ones column -> denominators) the moving
    one, so each block costs 65 cycles and the softmax normalization
    becomes a per-partition reciprocal+scale fused into the PSUM drain.
  - y (token-major) is transposed back per 128x128 block on the PE and
    the projection runs as in the baseline (yT stationary, wp moving).
  - two passes over query halves (i < 1024, i >= 1024) bound es SBUF and
    let first-half projection tiles overlap second-half attention.
  - Engine budget: PE ~154us, ACT ~152us, DVE ~85us, Pool ~50us.
"""

import sys

sys.path.insert(0, "/opt/trn_rl_repo")

from collections import deque
from contextlib import ExitStack

import ml_dtypes
import numpy as np

import concourse.bass as bass
import concourse.mybir as mybir
import concourse.tile as tile
from concourse import bacc
from concourse import bass_utils

f32 = mybir.dt.float32
f32r = mybir.dt.float32r
bf16 = mybir.dt.bfloat16
fp8 = mybir.dt.float8e4
u32 = mybir.dt.uint32
EXP = mybir.ActivationFunctionType.Exp
MUL = mybir.AluOpType.mult
ADD = mybir.AluOpType.add
DR = mybir.MatmulPerfMode.DoubleRow

B, T, C, H, D = 4, 2048, 1024, 16, 64
HL = H // 2          # 8 heads per core
CL = HL * D          # 512 local feature width
P = 128
NJC = T // P         # 16 token chunks of 128

# Weights are pre-scaled by SC on the host so their hi/lo fp8 split stays in
# e4m3's normal range (w ~ N(0, 0.02^2) would otherwise land in subnormals
# where the lo residual quantizes to zero). q/k/v come out SC x too large;
# 1/SC^2 folds into the exp scale and 1/SC into w_proj.
SC = 32.0

# es strip column offsets (packed per head): pass 1 covers i in
# [128jc, 1024), pass 2 covers i in [max(128jc, 1024), 2048).
W1 = [1024 - 128 * jc for jc in range(8)]
O1 = [sum(W1[:jc]) for jc in range(8)]
W2 = [min(1024, 2048 - 128 * jc) for jc in range(16)]
O2 = [sum(W2[:jc]) for jc in range(16)]
ES_COLS = O2[15] + W2[15]  # 12800


def build_body(tc, aps):
    nc = tc.nc

    with ExitStack() as ctx:
        const = ctx.enter_context(tc.tile_pool(name="const", bufs=1))
        xq_pool = ctx.enter_context(tc.tile_pool(name="xq", bufs=1))
        wq_pool = ctx.enter_context(tc.tile_pool(name="wq", bufs=1))
        wv_pool = ctx.enter_context(tc.tile_pool(name="wv", bufs=1))
        qk8_pool = ctx.enter_context(tc.tile_pool(name="qk8", bufs=1))
        vaug_pool = ctx.enter_context(tc.tile_pool(name="vaug", bufs=1))
        es_pool = ctx.enter_context(tc.tile_pool(name="es", bufs=2))
        y2_pool = ctx.enter_context(tc.tile_pool(name="y2", bufs=2))
        yT_pool = ctx.enter_context(tc.tile_pool(name="yT", bufs=1))
        wp_pool = ctx.enter_context(tc.tile_pool(name="wp", bufs=1))
        ostg = ctx.enter_context(tc.tile_pool(name="ostg", bufs=3))
        rc_pool = ctx.enter_context(tc.tile_pool(name="rc", bufs=2))
        psA = ctx.enter_context(tc.tile_pool(name="psA", bufs=2, space="PSUM"))
        psS = ctx.enter_context(tc.tile_pool(name="psS", bufs=2, space="PSUM"))
        psV = ctx.enter_context(tc.tile_pool(name="psV", bufs=2, space="PSUM"))

        # startup DMAs spread across the SP/ACT/Pool issue queues so the
        # first qk chunk can start at ~7us instead of ~14us
        bqk_sb = const.tile([P, 8], f32)
        nc.sync.dma_start(bqk_sb[:], aps["bqk"][:])
        xh_sb = xq_pool.tile([P, 4, 2, T], fp8, name="xh_sb")
        xl_sb = xq_pool.tile([P, 4, 2, T], fp8, name="xl_sb")
        nc.sync.dma_start(xh_sb[:], aps["xh"][:])
        nc.gpsimd.dma_start(xl_sb[:], aps["xl"][:])
        wqh_sb = wq_pool.tile([P, 8, 4, 2, P], fp8, name="wqh_sb")
        wql_sb = wq_pool.tile([P, 8, 4, 2, P], fp8, name="wql_sb")
        nc.scalar.dma_start(wqh_sb[:], aps["wqh"][:])
        nc.scalar.dma_start(wql_sb[:], aps["wql"][:])
        wvh_sb = wv_pool.tile([P, 4, 2, CL], fp8, name="wvh_sb")
        wvl_sb = wv_pool.tile([P, 4, 2, CL], fp8, name="wvl_sb")
        nc.scalar.dma_start(wvh_sb[:], aps["wvh"][:])
        masks_sb = const.tile([P, P], bf16)
        nc.sync.dma_start(masks_sb[:], aps["masks"][:])
        nc.sync.dma_start(wvl_sb[:], aps["wvl"][:])
        bv_rep = const.tile([P, CL], f32)
        nc.sync.dma_start(bv_rep[:], aps["bv"][None, :].to_broadcast([P, CL]))
        ident_sb = const.tile([P, P], bf16)
        nc.sync.dma_start(ident_sb[:], aps["ident"][:])

        # q/k fp8 pair-plane tiles, one per head-pair u: plane 0 = data,
        # plane 1 = zeros (kills the second DoubleRow term at 64-deep K).
        q8 = [qk8_pool.tile([P, 2, T], fp8, name=f"q8_{u}") for u in range(4)]
        k8 = [qk8_pool.tile([P, 2, T], fp8, name=f"k8_{u}") for u in range(4)]
        for t in q8 + k8:
            nc.gpsimd.memset(t[:, 1, :].bitcast(u32), 0)

        vaug = vaug_pool.tile([P, NJC, HL, D + 1], bf16)
        nc.vector.memset(vaug[:, :, :, D : D + 1], 1.0)

        yT = yT_pool.tile([P, 4, T], bf16)
        wp_sb = wp_pool.tile([P, 4, C], bf16, name="wp_sb")

        # ---------------- work-unit emitters ----------------
        def qk_chunk(jq, tci):
            # 512 tokens of q (jq<4) or k (jq>=4) chunk -> fp8 store
            ps = psA.tile([P, 512], f32, tag="a")
            first = True
            for kc in range(4):
                for wsb, xsb in ((wqh_sb, xh_sb), (wqh_sb, xl_sb),
                                 (wql_sb, xh_sb)):
                    nc.tensor.matmul(
                        ps[:], wsb[:, jq, kc], xsb[:, kc, :, tci * 512 : tci * 512 + 512],
                        start=first, stop=(kc == 3 and wsb is wql_sb),
                        perf_mode=DR,
                    )
                    first = False
            dest = q8[jq] if jq < 4 else k8[jq - 4]
            nc.vector.tensor_scalar_add(
                dest[:, 0, tci * 512 : tci * 512 + 512], ps[:],
                bqk_sb[:, jq : jq + 1],
            )

        def v_chunk(jc):
            # 128 tokens of v for all 8 heads -> vaug bf16
            ps = psA.tile([P, 512], f32, tag="a")
            first = True
            for kc in range(4):
                for wsb, xsb in ((wvh_sb, xh_sb), (wvh_sb, xl_sb),
                                 (wvl_sb, xh_sb)):
                    nc.tensor.matmul(
                        ps[:], xsb[:, kc, :, jc * P : (jc + 1) * P],
                        wsb[:, kc],
                        start=first, stop=(kc == 3 and wsb is wvl_sb),
                        perf_mode=DR,
                    )
                    first = False
            nc.vector.tensor_tensor(
                vaug[:, jc, :, 0:D],
                ps[:].rearrange("p (h d) -> p h d", h=HL),
                bv_rep[:].rearrange("p (h d) -> p h d", h=HL), ADD,
            )

        def s_strip(h, pas, jc, es_t):
            u, ko = h // 2, 64 * (h % 2)
            i0 = 128 * jc if pas == 1 else max(128 * jc, 1024)
            w = (1024 if pas == 1 else 2048) - i0
            off = O1[jc] if pas == 1 else O2[jc]
            ps = psS.tile([P, 1024], f32, tag="s")
            for c0 in range(0, w, 512):
                n = min(512, w - c0)
                nc.tensor.matmul(
                    ps[:, c0 : c0 + n],
                    k8[u][ko : ko + 64, :, jc * P : (jc + 1) * P],
                    q8[u][ko : ko + 64, :, i0 + c0 : i0 + c0 + n],
                    start=True, stop=True, perf_mode=DR,
                )
            nc.scalar.activation(
                es_t[:, off : off + w], ps[:, 0:w], EXP, scale=0.125 / (SC * SC)
            )
            if pas == 1 or jc >= 8:
                nc.gpsimd.tensor_tensor(
                    es_t[:, off : off + P], es_t[:, off : off + P], masks_sb[:], MUL
                )

        def av_ib(h, pas, ib, es_t, y2t):
            ps = psV.tile([P, 512], f32, tag="av")
            for jc in range(ib + 1):
                if pas == 1:
                    col = O1[jc] + (ib - jc) * P
                else:
                    col = O2[jc] + ib * P - max(128 * jc, 1024)
                nc.tensor.matmul(
                    ps[:, 0 : D + 1],
                    es_t[:, col : col + P],
                    vaug[:, jc, h, :],
                    start=(jc == 0), stop=(jc == ib),
                )
            rc = rc_pool.tile([P, 1], f32, tag="rc")
            nc.vector.reciprocal(rc[:], ps[:, D : D + 1])
            nc.vector.tensor_scalar_mul(
                y2t[:, ib % 8, 64 * (h % 2) : 64 * (h % 2) + 64], ps[:, 0:D], rc[:]
            )

        def transpose_one(u, pas, r, y2t):
            base = 0 if pas == 1 else 8
            pt = psV.tile([P, 512], f32, tag="av")
            ptb = pt[:, 0:64].bitcast(bf16)
            nc.tensor.transpose(ptb, y2t[:, r, :], ident_sb[:])
            nc.vector.tensor_copy(
                yT[:, u, (base + r) * P : (base + r + 1) * P], ptb
            )

        def c_tile(tcb, oc):
            ps = psA.tile([P, 512], f32, tag="a")
            for lc in range(4):
                nc.tensor.matmul(
                    ps[:],
                    yT[:, lc, tcb * P : (tcb + 1) * P],
                    wp_sb[:, lc, oc * 512 : oc * 512 + 512],
                    start=(lc == 0), stop=(lc == 3),
                )
            ot = ostg.tile([P, 512], f32, tag="o")
            nc.vector.tensor_copy(ot[:], ps[:])
            nc.sync.dma_start(
                aps["outp"][tcb * P : (tcb + 1) * P, oc * 512 : oc * 512 + 512],
                ot[:],
            )

        # ---------------- schedule ----------------
        filler = deque()
        state = {"done": 0}

        def need(k):
            while filler and state["done"] < k:
                filler.popleft()()
                state["done"] += 1

        def drip(n=1):
            for _ in range(n):
                if filler:
                    filler.popleft()()
                    state["done"] += 1

        # head-pair 0 q/k emitted directly; everything else via fillers.
        # v chunks 8..15 are deferred past the qk chunks: pass-1 AV only
        # reads v[jc<8], and pass-2 has idle PE while ACT churns exp.
        for tci in range(4):
            qk_chunk(4, tci)
            qk_chunk(0, tci)
        filler.extend(lambda jc=jc: v_chunk(jc) for jc in range(8))        # 0..7
        for grp in ((1, 5), (2, 6), (3, 7)):                               # 8..31
            filler.extend(
                lambda jq=jq, tci=tci: qk_chunk(jq, tci)
                for jq in grp for tci in range(4)
            )
        filler.extend(lambda jc=jc: v_chunk(jc) for jc in range(8, NJC))   # 32..39

        # Heads are software-pipelined: head h's AV/normalize/transpose work
        # (prev_work) executes interleaved into head h+1's strip loop so the
        # ACT exp stream never waits on a post-strip block. The last head of
        # each pass instead emits its own AV (and transposes/c_tiles) right
        # after the strip that completes each i-block, collapsing the tail.
        prev_work = deque()
        y2t_box = {}
        for pas in (1, 2):
            for h in range(HL):
                u, last = h // 2, h == HL - 1
                es_t = es_pool.tile([P, ES_COLS], bf16, tag="es", name=f"es{pas}_{h}")
                if h % 2 == 0:
                    y2t_box[(pas, u)] = y2_pool.tile(
                        [P, 8, P], bf16, tag="y2", name=f"y2{pas}_{u}"
                    )
                y2t = y2t_box[(pas, u)]
                if pas == 1 and u > 0:
                    need(8 + 8 * u)
                njc = 8 if pas == 1 else 16
                nslot = njc if not last else (8 if pas == 1 else 8)
                per = -(-len(prev_work) // nslot) if prev_work else 0

                def av_item(ib, h=h, pas=pas, es_t=es_t, y2t=y2t):
                    if pas == 1:
                        need(min(ib, 7) + 1 if h == 0 else 8)
                    else:
                        need(40)
                    av_ib(h, pas, ib, es_t, y2t)

                for jc in range(njc):
                    s_strip(h, pas, jc, es_t)
                    for _ in range(per):
                        if prev_work:
                            prev_work.popleft()()
                    if last and (pas == 1 or jc >= 8):
                        # i-block jc is complete for this, the last head
                        av_item(jc)
                        if pas == 2:
                            transpose_one(u, pas, jc - 8, y2t)
                            c_tile(jc, 0)
                            c_tile(jc, 1)
                    drip(1)
                while prev_work:
                    prev_work.popleft()()
                if last and pas == 1:
                    for r in range(8):
                        transpose_one(u, pas, r, y2t)
                else:
                    for ib in (range(8) if pas == 1 else range(8, 16)):
                        prev_work.append(lambda ib=ib, f=av_item: f(ib))
                    if h % 2 == 1 and not last:
                        prev_work.extend(
                            lambda u=u, pas=pas, r=r, y2t=y2t: transpose_one(
                                u, pas, r, y2t
                            )
                            for r in range(8)
                        )
                if pas == 1 and h == 0:
                    nc.sync.dma_start(
                        wp_sb[:], aps["wp"].rearrange("(l p) n -> p l n", p=P)
                    )
            if pas == 1:
                filler.extend(                                             # 40..55
                    lambda t=t, o=o: c_tile(t, o)
                    for t in range(8) for o in range(2)
                )
        while filler:
            filler.popleft()()


_CACHE = {}


def build_nc():
    if "nc" in _CACHE:
        return _CACHE["nc"]
    nc = bacc.Bacc(
        "TRN2",
        target_bir_lowering=False,
        debug=False,
        enable_asserts=False,
        num_devices=8,
    )
    aps = {
        "xh": nc.dram_tensor("xh", [P, 4, 2, T], fp8, kind="ExternalInput").ap(),
        "xl": nc.dram_tensor("xl", [P, 4, 2, T], fp8, kind="ExternalInput").ap(),
        "wqh": nc.dram_tensor("wqh", [P, 8, 4, 2, P], fp8, kind="ExternalInput").ap(),
        "wql": nc.dram_tensor("wql", [P, 8, 4, 2, P], fp8, kind="ExternalInput").ap(),
        "wvh": nc.dram_tensor("wvh", [P, 4, 2, CL], fp8, kind="ExternalInput").ap(),
        "wvl": nc.dram_tensor("wvl", [P, 4, 2, CL], fp8, kind="ExternalInput").ap(),
        "bqk": nc.dram_tensor("bqk", [P, 8], f32, kind="ExternalInput").ap(),
        "bv": nc.dram_tensor("bv", [CL], f32, kind="ExternalInput").ap(),
        "wp": nc.dram_tensor("wp", [CL, C], bf16, kind="ExternalInput").ap(),
        "masks": nc.dram_tensor("masks", [P, P], bf16, kind="ExternalInput").ap(),
        "ident": nc.dram_tensor("ident", [P, P], bf16, kind="ExternalInput").ap(),
        "outp": nc.dram_tensor("outp", [T, C], f32, kind="ExternalOutput").ap(),
    }
    with tile.TileContext(nc) as tc:
        build_body(tc, aps)
    nc.compile()
    _CACHE["nc"] = nc
    return nc


F8NP = mybir.dt.np(fp8)


def _hi_lo(a):
    hi = a.astype(F8NP)
    lo = (a - hi.astype(np.float32)).astype(F8NP)
    return hi, lo


def _dr_layout(a, free_shape):
    # [C, N...] with contraction c = kc*256 + i*128 + p -> [128, 4, 2, N...]
    return np.ascontiguousarray(
        a.reshape(4, 2, P, *free_shape).transpose(2, 0, 1, 3)
    )


def make_in_maps(x, w_attn, b_attn, w_proj, b_proj):
    masks = np.triu(np.ones((P, P), dtype=np.float32)).astype(ml_dtypes.bfloat16)
    ident = np.eye(P, dtype=np.float32).astype(ml_dtypes.bfloat16)
    in_maps = []
    for core in range(8):
        b, g = core // 2, core % 2
        xT = np.ascontiguousarray(x[b].T)  # [C, T]
        xh, xl = _hi_lo(xT)
        qcols = slice(g * CL, (g + 1) * CL)
        kcols = slice(C + g * CL, C + (g + 1) * CL)
        vcols = slice(2 * C + g * CL, 2 * C + (g + 1) * CL)
        wqk = SC * np.concatenate([w_attn[:, qcols], w_attn[:, kcols]], axis=1)
        wqh, wql = _hi_lo(wqk)
        wvh, wvl = _hi_lo(SC * w_attn[:, vcols])
        bqk = SC * np.concatenate([b_attn[qcols], b_attn[kcols]]).reshape(8, P).T
        in_maps.append(
            {
                "xh": _dr_layout(xh, (T,)),
                "xl": _dr_layout(xl, (T,)),
                # [C, 1024] -> [4, 2, 128p, 8jq, 128m] -> [p, jq, kc, i, m]
                "wqh": np.ascontiguousarray(
                    wqh.reshape(4, 2, P, 8, P).transpose(2, 3, 0, 1, 4)
                ),
                "wql": np.ascontiguousarray(
                    wql.reshape(4, 2, P, 8, P).transpose(2, 3, 0, 1, 4)
                ),
                "wvh": _dr_layout(wvh, (CL,)),
                "wvl": _dr_layout(wvl, (CL,)),
                "bqk": np.ascontiguousarray(bqk),
                "bv": np.ascontiguousarray(SC * b_attn[vcols]),
                "wp": np.ascontiguousarray(
                    (w_proj[g * CL : (g + 1) * CL, :] / SC).astype(ml_dtypes.bfloat16)
                ),
                "masks": masks,
                "ident": ident,
            }
        )
    return in_maps


def combine(parts, b_proj):
    return np.stack(
        [parts[2 * b] + parts[2 * b + 1] + b_proj[None, :] for b in range(B)]
    ).astype(np.float32)


def kernel(x, w_attn, b_attn, w_proj, b_proj, _trace=False, **run_kwargs):
    x = np.asarray(x, dtype=np.float32)
    w_attn = np.asarray(w_attn, dtype=np.float32)
    b_attn = np.asarray(b_attn, dtype=np.float32)
    w_proj = np.asarray(w_proj, dtype=np.float32)
    b_proj = np.asarray(b_proj, dtype=np.float32)

    nc = build_nc()
    in_maps = make_in_maps(x, w_attn, b_attn, w_proj, b_proj)
    try:
        res = bass_utils.run_bass_kernel_spmd(
            nc, in_maps, core_ids=list(range(8)), trace=_trace, **run_kwargs
        )
    except Exception:
        # transient NRT device wedge: one retry
        res = bass_utils.run_bass_kernel_spmd(
            nc, in_maps, core_ids=list(range(8)), trace=_trace, **run_kwargs
        )
    parts = [res.results[i]["outp"] for i in range(8)]
    out = combine(parts, b_proj)
    if _trace:
        return out, res
    return out
